# revision 7
# baseline (speedup 1.0000x reference)
"""Trainium2 Bass kernel for nn_Exchange_11055245820589 (gnn_message_passing).

reference computes:
    nodes = emb[z]                       # [N, 128]
    adj   = radius_graph_mask(pos, batch, 8.0)   # [N, N] bool
    h     = silu(nodes @ w1 + b1)        # [N, 64]
    out   = h @ w2 + b2                  # [N, 1]
    return out, adj

Structure exploited:
- `batch` is sorted, so each graph's nodes are contiguous and `adj` is
  block-diagonal: all true entries live in per-graph diagonal blocks
  (~64 blocks of ~256^2 out of 16384^2). The device computes only those
  blocks; the host scatters them into a calloc'd full bool array.
- `out[i]` depends only on `z[i]`: the whole MLP collapses to a 100-entry
  table (computed on device from emb/w1/b1/w2/b2) plus a gather, done on
  device via one-hot matmuls.

Sharding: data-parallel over graphs — 8 graphs per core, 8 cores. Each
core's inputs are its graphs' padded position blocks plus the replicated
tiny weights. SPMD: one program, per-core input maps.

Numerics: the grading reference runs on CPU XLA (fp32; matmul = fma-chain,
first product rounded). The device computes d2 in fp32 on the PE
(|d2_dev - d2_cpu| measured < 2e-3) and also emits a "band" mask
|d2 - 64| < 0.5; the host recomputes the handful of near-threshold pairs
with a bit-exact CPU-XLA emulation, making the final adj exactly equal to
the reference's.
"""
import numpy as np
from contextlib import ExitStack

import concourse.bacc as bacc
import concourse.mybir as mybir
import concourse.tile as tile
from concourse.bass_utils import run_bass_kernel_spmd

F32 = mybir.dt.float32
U8 = mybir.dt.uint8

N_CORES = 8
B_GRAPHS = 64
GPC = B_GRAPHS // N_CORES  # graphs per core
RADIUS2 = 64.0
PAD_POS = 1.0e6  # pad coordinate: d2 vs anything real is >> 64

_NC_CACHE = {}


def _build_nc(GP, NT, VOCAB, L0, H):
    """One NeuronCore program: GPC graph blocks of GP nodes + table MLP +
    one-hot gather for NT*128 nodes."""
    RT = GP // 128
    nc = bacc.Bacc("TRN2", target_bir_lowering=False, debug=False)

    posT_d = nc.declare_dram_parameter("posT", [GPC, 3, GP], F32, isOutput=False)
    posn_d = nc.declare_dram_parameter("posn", [GPC, GP, 3], F32, isOutput=False)
    zf_d = nc.declare_dram_parameter("zf", [1, NT * 128], F32, isOutput=False)
    embT_d = nc.declare_dram_parameter("embT", [L0, VOCAB], F32, isOutput=False)
    w1_d = nc.declare_dram_parameter("w1", [L0, H], F32, isOutput=False)
    b1_d = nc.declare_dram_parameter("b1", [H, 1], F32, isOutput=False)
    w2_d = nc.declare_dram_parameter("w2", [H, 1], F32, isOutput=False)
    b2_d = nc.declare_dram_parameter("b2", [1, 1], F32, isOutput=False)

    adj_d = nc.declare_dram_parameter("adj_blk", [GPC, GP, GP], U8, isOutput=True)
    # per-row count of near-threshold (|d2-64|<0.5) pairs; host re-verifies
    # any flagged row with the exact CPU-XLA emulation
    band_d = nc.declare_dram_parameter("band_rows", [GPC, 128, RT], F32, isOutput=True)
    outv_d = nc.declare_dram_parameter("outv", [128, NT], F32, isOutput=True)

    iota_d = nc.inline_tensor(
        np.arange(VOCAB, dtype=np.float32).reshape(VOCAB, 1), name="iota_vocab"
    )

    with tile.TileContext(nc) as tc, ExitStack() as ctx:
        consts = ctx.enter_context(tc.tile_pool(name="consts", bufs=1))
        gwork = ctx.enter_context(tc.tile_pool(name="gwork", bufs=2))
        work = ctx.enter_context(tc.tile_pool(name="work", bufs=3))
        psum = ctx.enter_context(tc.tile_pool(name="psum", bufs=2, space="PSUM"))
        psum_s = ctx.enter_context(tc.tile_pool(name="psum_s", bufs=1, space="PSUM"))
        psum_o = ctx.enter_context(tc.tile_pool(name="psum_o", bufs=2, space="PSUM"))

        iota_sb = consts.tile([VOCAB, 1], F32)
        nc.sync.dma_start(iota_sb[:], iota_d[:])
        ones3_sb = consts.tile([3, 128], F32)
        nc.vector.memset(ones3_sb[:], 1.0)
        ones1_sb = consts.tile([1, VOCAB], F32)
        nc.vector.memset(ones1_sb[:], 1.0)
        neg64_sb = consts.tile([128, 1], F32)
        nc.vector.memset(neg64_sb[:], -64.0)

        # ---------------- MLP table: tbl[v] = silu(emb @ w1 + b1) @ w2 + b2
        embT_sb = consts.tile([L0, VOCAB], F32)
        nc.sync.dma_start(embT_sb[:], embT_d[:])
        w1_sb = consts.tile([L0, H], F32)
        nc.sync.dma_start(w1_sb[:], w1_d[:])
        b1_sb = consts.tile([H, 1], F32)
        nc.sync.dma_start(b1_sb[:], b1_d[:])
        w2_sb = consts.tile([H, 1], F32)
        nc.sync.dma_start(w2_sb[:], w2_d[:])
        b2_sb = consts.tile([1, 1], F32)
        nc.sync.dma_start(b2_sb[:], b2_d[:])

        # bounce matmul inputs through DVE so each matmul waits on one sem
        embT_c = consts.tile([L0, VOCAB], F32)
        nc.vector.tensor_copy(embT_c[:], embT_sb[:])
        w1_c = consts.tile([L0, H], F32)
        nc.vector.tensor_copy(w1_c[:], w1_sb[:])
        w2_c = consts.tile([H, 1], F32)
        nc.vector.tensor_copy(w2_c[:], w2_sb[:])
        b2_c = consts.tile([1, 1], F32)
        nc.vector.tensor_copy(b2_c[:], b2_sb[:])

        h1p = psum_s.tile([H, VOCAB], F32, tag="h1p")
        nc.tensor.matmul(h1p[:], w1_c[:], embT_c[:], start=True, stop=True)
        hts = consts.tile([H, VOCAB], F32)
        nc.scalar.activation(
            hts[:], h1p[:], mybir.ActivationFunctionType.Silu, bias=b1_sb[:, 0:1]
        )
        hts_c = consts.tile([H, VOCAB], F32)
        nc.vector.tensor_copy(hts_c[:], hts[:])
        t1p = psum_s.tile([VOCAB, 1], F32, tag="t1p")
        nc.tensor.matmul(t1p[:], hts_c[:], w2_c[:], start=True, stop=False)
        nc.tensor.matmul(t1p[:], ones1_sb[:], b2_c[:], start=False, stop=True)
        tbl_sb = consts.tile([VOCAB, 1], F32)
        nc.vector.tensor_copy(tbl_sb[:], t1p[:])

        # ---------------- out gather: outv[p, t] = tbl[z[t*128+p]]
        zf_sb = consts.tile([1, NT * 128], F32)
        nc.sync.dma_start(zf_sb[:], zf_d[:])
        outs_sb = consts.tile([128, NT], F32)
        for t in range(NT):
            zb = work.tile([VOCAB, 128], F32, tag="zb")
            nc.gpsimd.partition_broadcast(zb[:], zf_sb[0:1, t * 128 : (t + 1) * 128])
            oh = work.tile([VOCAB, 128], F32, tag="oh")
            nc.vector.tensor_scalar(
                oh[:], zb[:], iota_sb[:, 0:1], None, mybir.AluOpType.is_equal
            )
            ovp = psum_o.tile([128, 1], F32, tag="ovp")
            nc.tensor.matmul(ovp[:], oh[:], tbl_sb[:], start=True, stop=True)
            nc.scalar.copy(outs_sb[:, t : t + 1], ovp[:])
        nc.sync.dma_start(outv_d[:], outs_sb[:])

        # ---------------- adjacency blocks
        for g in range(GPC):
            posT_sb = gwork.tile([3, GP], F32, tag="posT")
            nc.sync.dma_start(posT_sb[:], posT_d[g])
            posT2_sb = gwork.tile([3, GP], F32, tag="posT2")
            nc.vector.tensor_mul(posT2_sb[:], posT_sb[:], posT_sb[:])
            # sqj[p, j] = |pos_j|^2 for every partition p (ones-matmul bcast)
            sqj_ps = psum.tile([128, GP], F32, tag="sqj")
            nc.tensor.matmul(
                sqj_ps[:], ones3_sb[:], posT2_sb[:], start=True, stop=True
            )
            # per-row norms |pos_i|^2 as per-partition scalars
            pn = gwork.tile([128, RT, 3], F32, tag="pn")
            nc.sync.dma_start(
                pn[:], posn_d[g].rearrange("(r p) c -> p r c", p=128)
            )
            p2 = gwork.tile([128, RT, 3], F32, tag="p2")
            nc.vector.tensor_mul(p2[:], pn[:], pn[:])
            sq_sb = gwork.tile([128, RT], F32, tag="sq")
            nc.vector.reduce_sum(sq_sb[:], p2[:], axis=mybir.AxisListType.X)

            stage = gwork.tile([128, RT, GP], U8, tag="stage")
            brow = gwork.tile([128, RT], F32, tag="brow")
            for r in range(RT):
                rows = slice(r * 128, (r + 1) * 128)
                gm = psum.tile([128, GP], F32, tag="gm")
                nc.tensor.matmul(
                    gm[:], posT_sb[:, rows], posT_sb[:], start=True, stop=True
                )
                r2_sb = work.tile([128, GP], F32, tag="r2")
                nc.scalar.mul(r2_sb[:], gm[:], 2.0)  # exact
                d2_sb = work.tile([128, GP], F32, tag="d2")
                nc.vector.scalar_tensor_tensor(
                    d2_sb[:],
                    sqj_ps[:],
                    sq_sb[:, r : r + 1],
                    r2_sb[:],
                    op0=mybir.AluOpType.add,
                    op1=mybir.AluOpType.subtract,
                )
                nc.gpsimd.tensor_scalar(
                    stage[:, r, :], d2_sb[:], RADIUS2, None, mybir.AluOpType.is_lt
                )
                ab_sb = work.tile([128, GP], F32, tag="ab")
                nc.scalar.activation(
                    ab_sb[:],
                    d2_sb[:],
                    mybir.ActivationFunctionType.Abs,
                    bias=neg64_sb[:, 0:1],
                )
                bd = work.tile([128, GP], U8, tag="bd")
                nc.vector.tensor_scalar(
                    bd[:],
                    ab_sb[:],
                    0.5,
                    0.0,
                    mybir.AluOpType.is_lt,
                    op1=mybir.AluOpType.add,
                    accum_out=brow[:, r : r + 1],
                )
            nc.sync.dma_start(
                adj_d[g].rearrange("(r p) c -> p r c", p=128), stage[:]
            )
            nc.sync.dma_start(band_d[g], brow[:])

    nc.finalize()
    return nc


def _cpu_xla_d2(a, b):
    """Bit-exact emulation of the reference's fp32 d2 on CPU XLA for pair
    arrays a, b of shape [k, 3] (float32).

    G: first product rounded to f32, then fma steps (exact product + add,
    single rounding). sq: sequentially rounded products/adds."""
    a = a.astype(np.float32)
    b = b.astype(np.float32)
    a64 = a.astype(np.float64)
    b64 = b.astype(np.float64)
    g = (a[:, 0] * b[:, 0]).astype(np.float32)
    g = (a64[:, 1] * b64[:, 1] + g.astype(np.float64)).astype(np.float32)
    g = (a64[:, 2] * b64[:, 2] + g.astype(np.float64)).astype(np.float32)

    def sq(v):
        return ((v[:, 0] * v[:, 0] + v[:, 1] * v[:, 1]).astype(np.float32)
                + v[:, 2] * v[:, 2]).astype(np.float32)

    t1 = (sq(a) + sq(b)).astype(np.float32)
    return (t1 - (np.float32(2.0) * g).astype(np.float32)).astype(np.float32)


def _prepare(z, batch, pos, emb, w1, b1, w2, b2):
    z = np.asarray(z)
    batch = np.asarray(batch)
    pos = np.ascontiguousarray(np.asarray(pos), dtype=np.float32)
    emb = np.asarray(emb, dtype=np.float32)
    w1 = np.asarray(w1, dtype=np.float32)
    b1 = np.asarray(b1, dtype=np.float32)
    w2 = np.asarray(w2, dtype=np.float32)
    b2 = np.asarray(b2, dtype=np.float32)

    N = z.shape[0]
    VOCAB, L0 = emb.shape
    H = w1.shape[1]

    ids = np.arange(B_GRAPHS)
    starts = np.searchsorted(batch, ids, "left").astype(np.int64)
    ends = np.searchsorted(batch, ids, "right").astype(np.int64)
    sizes = ends - starts
    GP = int(max(128, -(-int(sizes.max()) // 128) * 128))

    core_start = starts[0::GPC]
    core_end = ends[GPC - 1 :: GPC]
    core_cnt = core_end - core_start
    NP = int(max(128, -(-int(core_cnt.max()) // 128) * 128))
    NT = NP // 128

    in_maps = []
    for m in range(N_CORES):
        posT = np.empty((GPC, 3, GP), dtype=np.float32)
        posn = np.full((GPC, GP, 3), PAD_POS, dtype=np.float32)
        for gl in range(GPC):
            g = m * GPC + gl
            s, e = int(starts[g]), int(ends[g])
            posn[gl, : e - s] = pos[s:e]
            posT[gl] = posn[gl].T
        zf = np.zeros((1, NP), dtype=np.float32)
        zf[0, : int(core_cnt[m])] = z[int(core_start[m]) : int(core_end[m])]
        in_maps.append(
            {
                "posT": posT,
                "posn": posn,
                "zf": zf,
                "embT": np.ascontiguousarray(emb.T),
                "w1": w1,
                "b1": b1.reshape(H, 1),
                "w2": w2.reshape(H, 1),
                "b2": b2.reshape(1, 1),
            }
        )

    meta = {
        "N": N,
        "GP": GP,
        "NT": NT,
        "VOCAB": VOCAB,
        "L0": L0,
        "H": H,
        "starts": starts,
        "ends": ends,
        "core_start": core_start,
        "core_cnt": core_cnt,
        "pos": pos,
    }
    return in_maps, meta


def _get_nc(meta):
    key = (meta["GP"], meta["NT"], meta["VOCAB"], meta["L0"], meta["H"])
    nc = _NC_CACHE.get(key)
    if nc is None:
        nc = _build_nc(*key)
        _NC_CACHE[key] = nc
    return nc


def _assemble(results, meta):
    N = meta["N"]
    starts, ends = meta["starts"], meta["ends"]
    core_start, core_cnt = meta["core_start"], meta["core_cnt"]
    pos = meta["pos"]

    out = np.empty((N, 1), dtype=np.float32)
    adj = np.zeros((N, N), dtype=bool)
    for m in range(N_CORES):
        r = results[m]
        vals = r["outv"].T.ravel()[: int(core_cnt[m])]
        cs = int(core_start[m])
        out[cs : cs + int(core_cnt[m]), 0] = vals
        for gl in range(GPC):
            g = m * GPC + gl
            s, e = int(starts[g]), int(ends[g])
            n = e - s
            if n == 0:
                continue
            blk = r["adj_blk"][gl][:n, :n].astype(bool)
            # device counted near-threshold pairs per row; recompute any
            # flagged row with the exact CPU-XLA fp32 emulation
            br = r["band_rows"][gl]  # [128, RT], row (r*128+p) count at [p, r]
            rows = np.nonzero(br.T.ravel() > 0)[0]
            rows = rows[rows < n]
            if rows.size:
                P = pos[s:e]
                a = np.repeat(P[rows], n, axis=0)
                b = np.tile(P, (rows.size, 1))
                d2 = _cpu_xla_d2(a, b).reshape(rows.size, n)
                blk[rows] = d2 < np.float32(RADIUS2)
            np.fill_diagonal(blk, False)
            adj[s:e, s:e] = blk
    return out, adj


def kernel(**inputs):
    in_maps, meta = _prepare(**inputs)
    nc = _get_nc(meta)
    res = run_bass_kernel_spmd(nc, in_maps, list(range(N_CORES)))
    return _assemble(res.results, meta)


# revision 18
# speedup vs baseline: 1.2971x; 1.2971x over previous
"""Trainium2 Bass kernel for nn_Exchange_11055245820589 (gnn_message_passing).

reference computes:
    nodes = emb[z]                       # [N, 128]
    adj   = radius_graph_mask(pos, batch, 8.0)   # [N, N] bool
    h     = silu(nodes @ w1 + b1)        # [N, 64]
    out   = h @ w2 + b2                  # [N, 1]
    return out, adj

Structure exploited:
- `batch` is sorted, so each graph's nodes are contiguous and `adj` is
  block-diagonal: all true entries live in per-graph diagonal blocks
  (~64 blocks of ~256^2 out of 16384^2). The device computes only those
  blocks; the host scatters them into a calloc'd full bool array.
- `out[i]` depends only on `z[i]`: the whole MLP collapses to a 100-entry
  table (computed on device from emb/w1/b1/w2/b2) plus a gather, done on
  device via one-hot matmuls.

Sharding: data-parallel over graphs — 8 graphs per core, 8 cores, one SPMD
program. Graphs are sorted by size and packed into a per-core SLOT PROFILE
(NBIG slots padded to GP_BIG, NSML slots padded to GP_SML) that is identical
on every core, so small graphs don't pay the big-graph padding.

Numerics: the grading reference runs on CPU XLA (fp32; matmul = fma-chain,
first product rounded). The device computes d2 in fp32 on the PE
(|d2_dev - d2_cpu| measured < 2e-3) and counts near-threshold pairs
(|d2-64| < 0.5) per row; the host recomputes flagged rows with a bit-exact
CPU-XLA emulation, making the final adj exactly equal to the reference's.
"""
import numpy as np
from contextlib import ExitStack

import concourse.bacc as bacc
import concourse.mybir as mybir
import concourse.tile as tile
from concourse.bass_utils import run_bass_kernel_spmd

F32 = mybir.dt.float32
U8 = mybir.dt.uint8

N_CORES = 8
B_GRAPHS = 64
GPC = B_GRAPHS // N_CORES  # graph slots per core
RADIUS2 = 64.0
PAD_POS = 1.0e6  # pad coordinate: d2 vs anything real is >> 64

_NC_CACHE = {}


def _build_nc(profile, NT, VOCAB, L0, H):
    """One NeuronCore program. `profile` = per-slot padded graph sizes
    (multiples of 128), identical on every core."""
    rts = [-(-gp // 128) for gp in profile]
    TOTC = sum(profile)                     # total posT columns
    TOTN = sum(rt * 128 for rt in rts)      # total posn rows (row-padded)
    TOTB = sum(rt * 128 * gp for rt, gp in zip(rts, profile))  # adj bytes
    MAXRT = max(rts)
    nc = bacc.Bacc("TRN2", target_bir_lowering=False, debug=False)

    posT_d = nc.declare_dram_parameter("posT", [3, TOTC], F32, isOutput=False)
    posn_d = nc.declare_dram_parameter("posn", [TOTN, 3], F32, isOutput=False)
    zf_d = nc.declare_dram_parameter("zf", [1, NT * 128], F32, isOutput=False)
    embT_d = nc.declare_dram_parameter("embT", [L0, VOCAB], F32, isOutput=False)
    w1_d = nc.declare_dram_parameter("w1", [L0, H], F32, isOutput=False)
    b1_d = nc.declare_dram_parameter("b1", [H, 1], F32, isOutput=False)
    w2_d = nc.declare_dram_parameter("w2", [H, 1], F32, isOutput=False)
    b2_d = nc.declare_dram_parameter("b2", [1, 1], F32, isOutput=False)

    adj_d = nc.declare_dram_parameter("adj_blk", [TOTB], U8, isOutput=True)
    # per-row count of near-threshold (|d2-64|<0.5) pairs; host re-verifies
    # any flagged row with the exact CPU-XLA emulation
    band_d = nc.declare_dram_parameter(
        "band_rows", [GPC, 128, MAXRT], F32, isOutput=True
    )
    outv_d = nc.declare_dram_parameter("outv", [128, NT], F32, isOutput=True)

    iota_d = nc.inline_tensor(
        np.arange(VOCAB, dtype=np.float32).reshape(VOCAB, 1), name="iota_vocab"
    )

    with tile.TileContext(nc) as tc, ExitStack() as ctx:
        consts = ctx.enter_context(tc.tile_pool(name="consts", bufs=1))
        gwork = ctx.enter_context(tc.tile_pool(name="gwork", bufs=3))
        work = ctx.enter_context(tc.tile_pool(name="work", bufs=6))
        psum = ctx.enter_context(tc.tile_pool(name="psum", bufs=3, space="PSUM"))
        psum_g = ctx.enter_context(tc.tile_pool(name="psum_g", bufs=2, space="PSUM"))
        psum_s = ctx.enter_context(tc.tile_pool(name="psum_s", bufs=1, space="PSUM"))
        psum_o = ctx.enter_context(tc.tile_pool(name="psum_o", bufs=1, space="PSUM"))

        iota_sb = consts.tile([VOCAB, 1], F32)
        nc.sync.dma_start(iota_sb[:], iota_d[:])
        ones3_sb = consts.tile([3, 128], F32)
        nc.vector.memset(ones3_sb[:], 1.0)
        ones1_sb = consts.tile([1, VOCAB], F32)
        nc.vector.memset(ones1_sb[:], 1.0)
        neg64_sb = consts.tile([128, 1], F32)
        nc.vector.memset(neg64_sb[:], -64.0)

        # ---------------- MLP table: tbl[v] = silu(emb @ w1 + b1) @ w2 + b2
        embT_sb = consts.tile([L0, VOCAB], F32)
        nc.sync.dma_start(embT_sb[:], embT_d[:])
        w1_sb = consts.tile([L0, H], F32)
        nc.sync.dma_start(w1_sb[:], w1_d[:])
        b1_sb = consts.tile([H, 1], F32)
        nc.sync.dma_start(b1_sb[:], b1_d[:])
        w2_sb = consts.tile([H, 1], F32)
        nc.sync.dma_start(w2_sb[:], w2_d[:])
        b2_sb = consts.tile([1, 1], F32)
        nc.sync.dma_start(b2_sb[:], b2_d[:])

        # bounce matmul inputs through DVE so each matmul waits on one sem
        embT_c = consts.tile([L0, VOCAB], F32)
        nc.vector.tensor_copy(embT_c[:], embT_sb[:])
        w1_c = consts.tile([L0, H], F32)
        nc.vector.tensor_copy(w1_c[:], w1_sb[:])
        w2_c = consts.tile([H, 1], F32)
        nc.vector.tensor_copy(w2_c[:], w2_sb[:])
        b2_c = consts.tile([1, 1], F32)
        nc.vector.tensor_copy(b2_c[:], b2_sb[:])

        h1p = psum_s.tile([H, VOCAB], F32, tag="h1p")
        nc.tensor.matmul(h1p[:], w1_c[:], embT_c[:], start=True, stop=True)
        hts = consts.tile([H, VOCAB], F32)
        nc.scalar.activation(
            hts[:], h1p[:], mybir.ActivationFunctionType.Silu, bias=b1_sb[:, 0:1]
        )
        hts_c = consts.tile([H, VOCAB], F32)
        nc.vector.tensor_copy(hts_c[:], hts[:])
        t1p = psum_s.tile([VOCAB, 1], F32, tag="t1p")
        nc.tensor.matmul(t1p[:], hts_c[:], w2_c[:], start=True, stop=False)
        nc.tensor.matmul(t1p[:], ones1_sb[:], b2_c[:], start=False, stop=True)
        tbl_sb = consts.tile([VOCAB, 1], F32)
        nc.vector.tensor_copy(tbl_sb[:], t1p[:])

        # ---------------- out gather: outv[p, t] = tbl[z[t*128+p]]
        zf_sb = consts.tile([1, NT * 128], F32)
        nc.sync.dma_start(zf_sb[:], zf_d[:])
        zb = consts.tile([VOCAB, NT * 128], F32)
        nc.gpsimd.partition_broadcast(zb[:], zf_sb[0:1, :])
        oh = consts.tile([VOCAB, NT * 128], F32)
        nc.vector.tensor_scalar(
            oh[:], zb[:], iota_sb[:, 0:1], None, mybir.AluOpType.is_equal
        )
        outs_sb = consts.tile([128, NT], F32)
        ovp = psum_o.tile([128, NT], F32, tag="ovp")
        for t in range(NT):
            nc.tensor.matmul(
                ovp[:, t : t + 1],
                oh[:, t * 128 : (t + 1) * 128],
                tbl_sb[:],
                start=True,
                stop=True,
            )
        nc.scalar.copy(outs_sb[:], ovp[:])
        nc.sync.dma_start(outv_d[:], outs_sb[:])

        # ---------------- adjacency blocks (per slot, mixed sizes)
        GPB = max(profile)
        coff = 0
        noff = 0
        boff = 0
        for g, GP in enumerate(profile):
            RT = -(-GP // 128)
            posT_sb = gwork.tile([3, GPB], F32, tag="posT")
            nc.sync.dma_start(posT_sb[:, :GP], posT_d[:, coff : coff + GP])
            posT2_sb = gwork.tile([3, GPB], F32, tag="posT2")
            nc.vector.tensor_mul(posT2_sb[:, :GP], posT_sb[:, :GP], posT_sb[:, :GP])
            # 2*posT so the G matmul yields 2G directly (band absorbs rounding)
            posTx2_sb = gwork.tile([3, GPB], F32, tag="posTx2")
            nc.vector.tensor_scalar(
                posTx2_sb[:, :GP], posT_sb[:, :GP], 2.0, None, mybir.AluOpType.mult
            )
            # sqj[p, j] = |pos_j|^2 for every partition p (ones-matmul bcast)
            sqj_ps = psum_g.tile([128, GPB], F32, tag="sqj")
            nc.tensor.matmul(
                sqj_ps[:, :GP], ones3_sb[:], posT2_sb[:, :GP], start=True, stop=True
            )
            sqj_sb = gwork.tile([128, GPB], F32, tag="sqj_sb")
            nc.scalar.copy(sqj_sb[:, :GP], sqj_ps[:, :GP])
            # per-row norms |pos_i|^2 as per-partition scalars
            pn = gwork.tile([128, MAXRT, 3], F32, tag="pn")
            nc.sync.dma_start(
                pn[:, :RT, :],
                posn_d[noff : noff + RT * 128].rearrange("(r p) c -> p r c", p=128),
            )
            p2 = gwork.tile([128, MAXRT, 3], F32, tag="p2")
            nc.vector.tensor_mul(p2[:, :RT, :], pn[:, :RT, :], pn[:, :RT, :])
            sq_sb = gwork.tile([128, MAXRT], F32, tag="sq")
            nc.vector.reduce_sum(
                sq_sb[:, :RT], p2[:, :RT, :], axis=mybir.AxisListType.X
            )

            stage = gwork.tile([128, MAXRT, GPB], U8, tag="stage")
            brow = gwork.tile([128, MAXRT], F32, tag="brow")
            for r in range(RT):
                M = min(128, GP - r * 128)
                rows = slice(r * 128, r * 128 + M)
                gm = psum.tile([128, GPB], F32, tag="gm")
                nc.tensor.matmul(
                    gm[:M, :GP],
                    posTx2_sb[:, rows],
                    posT_sb[:, :GP],
                    start=True,
                    stop=True,
                )
                d2_sb = work.tile([128, GPB], F32, tag="d2")
                nc.vector.scalar_tensor_tensor(
                    d2_sb[:M, :GP],
                    sqj_sb[:M, :GP],
                    sq_sb[:M, r : r + 1],
                    gm[:M, :GP],
                    op0=mybir.AluOpType.add,
                    op1=mybir.AluOpType.subtract,
                )
                nc.gpsimd.tensor_scalar(
                    stage[:M, r, :GP], d2_sb[:M, :GP], RADIUS2, None,
                    mybir.AluOpType.is_lt,
                )
                ab_sb = work.tile([128, GPB], F32, tag="ab")
                nc.scalar.activation(
                    ab_sb[:M, :GP],
                    d2_sb[:M, :GP],
                    mybir.ActivationFunctionType.Abs,
                    bias=neg64_sb[:M, 0:1],
                )
                bd = work.tile([128, GPB], U8, tag="bd")
                nc.vector.tensor_scalar(
                    bd[:M, :GP],
                    ab_sb[:M, :GP],
                    0.5,
                    0.0,
                    mybir.AluOpType.is_lt,
                    op1=mybir.AluOpType.add,
                    accum_out=brow[:M, r : r + 1],
                )
            nc.sync.dma_start(
                adj_d[boff : boff + RT * 128 * GP].rearrange(
                    "(r p c) -> p r c", p=128, c=GP
                ),
                stage[:, :RT, :GP],
            )
            nc.sync.dma_start(band_d[g, :, :RT], brow[:, :RT])
            coff += GP
            noff += RT * 128
            boff += RT * 128 * GP

    nc.finalize()
    return nc


def _cpu_xla_d2(a, b):
    """Bit-exact emulation of the reference's fp32 d2 on CPU XLA for pair
    arrays a, b of shape [k, 3] (float32)."""
    a = a.astype(np.float32)
    b = b.astype(np.float32)
    a64 = a.astype(np.float64)
    b64 = b.astype(np.float64)
    g = (a[:, 0] * b[:, 0]).astype(np.float32)
    g = (a64[:, 1] * b64[:, 1] + g.astype(np.float64)).astype(np.float32)
    g = (a64[:, 2] * b64[:, 2] + g.astype(np.float64)).astype(np.float32)

    def sq(v):
        return ((v[:, 0] * v[:, 0] + v[:, 1] * v[:, 1]).astype(np.float32)
                + v[:, 2] * v[:, 2]).astype(np.float32)

    t1 = (sq(a) + sq(b)).astype(np.float32)
    return (t1 - (np.float32(2.0) * g).astype(np.float32)).astype(np.float32)


def _prepare(z, batch, pos, emb, w1, b1, w2, b2):
    z = np.asarray(z)
    batch = np.asarray(batch)
    pos = np.ascontiguousarray(np.asarray(pos), dtype=np.float32)
    emb = np.asarray(emb, dtype=np.float32)
    w1 = np.asarray(w1, dtype=np.float32)
    b1 = np.asarray(b1, dtype=np.float32)
    w2 = np.asarray(w2, dtype=np.float32)
    b2 = np.asarray(b2, dtype=np.float32)

    N = z.shape[0]
    VOCAB, L0 = emb.shape
    H = w1.shape[1]

    ids = np.arange(B_GRAPHS)
    starts = np.searchsorted(batch, ids, "left").astype(np.int64)
    ends = np.searchsorted(batch, ids, "right").astype(np.int64)
    sizes = (ends - starts).astype(np.int64)

    # slot profile: slot s (same on every core) holds graphs ranked
    # [s*8, s*8+8) by size, so its width is the max size in that rank band
    order = np.argsort(-sizes, kind="stable")  # graph ids, biggest first
    profile = tuple(
        int(max(1, sizes[order[s * N_CORES]])) for s in range(GPC)
    )

    # assignment: slot (core m, slot s) <- order[s*N_CORES + m]
    slot_graph = np.full((N_CORES, GPC), -1, dtype=np.int64)
    for s in range(GPC):
        for m in range(N_CORES):
            slot_graph[m, s] = order[s * N_CORES + m]

    core_cnt = np.array(
        [int(sizes[slot_graph[m]].sum()) for m in range(N_CORES)], dtype=np.int64
    )
    NP = int(max(128, -(-int(core_cnt.max()) // 128) * 128))
    NT = NP // 128
    TOTC = sum(profile)

    rts = [-(-gp // 128) for gp in profile]
    TOTN = sum(rt * 128 for rt in rts)
    in_maps = []
    for m in range(N_CORES):
        posT = np.full((3, TOTC), PAD_POS, dtype=np.float32)
        posn = np.full((TOTN, 3), PAD_POS, dtype=np.float32)
        zc = np.zeros(NP, dtype=np.float32)
        coff = 0
        noff = 0
        zoff = 0
        for s, GP in enumerate(profile):
            g = int(slot_graph[m, s])
            sg, eg = int(starts[g]), int(ends[g])
            ng = eg - sg
            posn[noff : noff + ng] = pos[sg:eg]
            posT[:, coff : coff + ng] = pos[sg:eg].T
            zc[zoff : zoff + ng] = z[sg:eg]
            coff += GP
            noff += rts[s] * 128
            zoff += ng
        in_maps.append(
            {
                "posT": posT,
                "posn": posn,
                "zf": zc.reshape(1, NP),
                "embT": np.ascontiguousarray(emb.T),
                "w1": w1,
                "b1": b1.reshape(H, 1),
                "w2": w2.reshape(H, 1),
                "b2": b2.reshape(1, 1),
            }
        )

    meta = {
        "N": N,
        "profile": profile,
        "NT": NT,
        "VOCAB": VOCAB,
        "L0": L0,
        "H": H,
        "starts": starts,
        "ends": ends,
        "slot_graph": slot_graph,
        "pos": pos,
    }
    return in_maps, meta


def _get_nc(meta):
    key = (meta["profile"], meta["NT"], meta["VOCAB"], meta["L0"], meta["H"])
    nc = _NC_CACHE.get(key)
    if nc is None:
        nc = _build_nc(*key)
        _NC_CACHE[key] = nc
    return nc


def _assemble(results, meta):
    N = meta["N"]
    starts, ends = meta["starts"], meta["ends"]
    slot_graph = meta["slot_graph"]
    profile = meta["profile"]
    pos = meta["pos"]

    out = np.empty((N, 1), dtype=np.float32)
    adj = np.zeros((N, N), dtype=bool)
    for m in range(N_CORES):
        r = results[m]
        outv = r["outv"].T.ravel()
        boff = 0
        zoff = 0
        for s, GP in enumerate(profile):
            RTs = -(-GP // 128)
            g = int(slot_graph[m, s])
            sg, eg = int(starts[g]), int(ends[g])
            n = eg - sg
            blk_full = r["adj_blk"][boff : boff + RTs * 128 * GP].reshape(
                RTs * 128, GP
            )
            boff += RTs * 128 * GP
            out[sg:eg, 0] = outv[zoff : zoff + n]
            zoff += n
            if n == 0:
                continue
            blk = blk_full[:n, :n].astype(bool)
            br = r["band_rows"][s]  # [128, MAXRT]
            RT = -(-GP // 128)
            rows = np.nonzero(br[:, :RT].T.ravel() > 0)[0]
            rows = rows[rows < n]
            if rows.size:
                P = pos[sg:eg]
                a = np.repeat(P[rows], n, axis=0)
                b = np.tile(P, (rows.size, 1))
                d2 = _cpu_xla_d2(a, b).reshape(rows.size, n)
                blk[rows] = d2 < np.float32(RADIUS2)
            np.fill_diagonal(blk, False)
            adj[sg:eg, sg:eg] = blk
    return out, adj


def kernel(**inputs):
    in_maps, meta = _prepare(**inputs)
    nc = _get_nc(meta)
    res = run_bass_kernel_spmd(nc, in_maps, list(range(N_CORES)))
    return _assemble(res.results, meta)


# revision 25
# speedup vs baseline: 1.4167x; 1.0922x over previous
"""Trainium2 Bass kernel for nn_Exchange_11055245820589 (gnn_message_passing).

reference computes:
    nodes = emb[z]                       # [N, 128]
    adj   = radius_graph_mask(pos, batch, 8.0)   # [N, N] bool
    h     = silu(nodes @ w1 + b1)        # [N, 64]
    out   = h @ w2 + b2                  # [N, 1]
    return out, adj

Structure exploited:
- `batch` is sorted, so each graph's nodes are contiguous and `adj` is
  block-diagonal: all true entries live in per-graph diagonal blocks
  (~64 blocks of ~256^2 out of 16384^2). The device computes only those
  blocks; the host scatters them into a calloc'd full bool array.
- `out[i]` depends only on `z[i]`: the whole MLP collapses to a 100-entry
  table (computed on device from emb/w1/b1/w2/b2) plus a gather, done on
  device via one-hot matmuls.

Sharding: data-parallel over graphs — 8 graphs per core, 8 cores, one SPMD
program. Graphs are sorted by size and packed into a per-core SLOT PROFILE
(NBIG slots padded to GP_BIG, NSML slots padded to GP_SML) that is identical
on every core, so small graphs don't pay the big-graph padding.

Numerics: the grading reference runs on CPU XLA (fp32; matmul = fma-chain,
first product rounded). The device computes d2 in fp32 on the PE
(|d2_dev - d2_cpu| measured < 2e-3) and counts near-threshold pairs
(|d2-64| < 0.5) per row; the host recomputes flagged rows with a bit-exact
CPU-XLA emulation, making the final adj exactly equal to the reference's.
"""
import numpy as np
from contextlib import ExitStack

import concourse.bacc as bacc
import concourse.mybir as mybir
import concourse.tile as tile
from concourse.bass_utils import run_bass_kernel_spmd

F32 = mybir.dt.float32
U8 = mybir.dt.uint8

N_CORES = 8
B_GRAPHS = 64
GPC = B_GRAPHS // N_CORES  # graph slots per core
RADIUS2 = 64.0
PAD_POS = 1.0e6  # pad coordinate: d2 vs anything real is >> 64

_NC_CACHE = {}


def _build_nc(profile, NT, VOCAB, L0, H):
    """One NeuronCore program. `profile` = per-slot padded graph sizes
    (multiples of 128), identical on every core."""
    rts = [-(-gp // 128) for gp in profile]
    TOTC = sum(profile)                     # total posT columns
    TOTN = sum(rt * 128 for rt in rts)      # total posn rows (row-padded)
    TOTB = sum(rt * 128 * gp for rt, gp in zip(rts, profile))  # adj bytes
    MAXRT = max(rts)
    nc = bacc.Bacc("TRN2", target_bir_lowering=False, debug=False)

    posT_d = nc.declare_dram_parameter("posT", [3, TOTC], F32, isOutput=False)
    posn_d = nc.declare_dram_parameter("posn", [TOTN, 3], F32, isOutput=False)
    zf_d = nc.declare_dram_parameter("zf", [1, NT * 128], F32, isOutput=False)
    embT_d = nc.declare_dram_parameter("embT", [L0, VOCAB], F32, isOutput=False)
    w1_d = nc.declare_dram_parameter("w1", [L0, H], F32, isOutput=False)
    b1_d = nc.declare_dram_parameter("b1", [H, 1], F32, isOutput=False)
    w2_d = nc.declare_dram_parameter("w2", [H, 1], F32, isOutput=False)
    b2_d = nc.declare_dram_parameter("b2", [1, 1], F32, isOutput=False)

    # two threshold planes: lo = (d2 < 63.5), hi = sign(64.5 - d2) in u8.
    # pairs where the two disagree are near-threshold; host recomputes them
    # with the exact CPU-XLA emulation
    adj_d = nc.declare_dram_parameter("adj_lo", [TOTB], U8, isOutput=True)
    adj2_d = nc.declare_dram_parameter("adj_hi", [TOTB], U8, isOutput=True)
    outv_d = nc.declare_dram_parameter("outv", [128, NT], F32, isOutput=True)

    iota_d = nc.inline_tensor(
        np.arange(VOCAB, dtype=np.float32).reshape(VOCAB, 1), name="iota_vocab"
    )

    with tile.TileContext(nc) as tc, ExitStack() as ctx:
        consts = ctx.enter_context(tc.tile_pool(name="consts", bufs=1))
        gwork = ctx.enter_context(tc.tile_pool(name="gwork", bufs=3))
        work = ctx.enter_context(tc.tile_pool(name="work", bufs=6))
        psum = ctx.enter_context(tc.tile_pool(name="psum", bufs=3, space="PSUM"))
        psum_g = ctx.enter_context(tc.tile_pool(name="psum_g", bufs=2, space="PSUM"))
        psum_s = ctx.enter_context(tc.tile_pool(name="psum_s", bufs=1, space="PSUM"))
        psum_o = ctx.enter_context(tc.tile_pool(name="psum_o", bufs=1, space="PSUM"))

        iota_sb = consts.tile([VOCAB, 1], F32)
        nc.sync.dma_start(iota_sb[:], iota_d[:])
        ones3_sb = consts.tile([3, 128], F32)
        nc.vector.memset(ones3_sb[:], 1.0)
        ones1_sb = consts.tile([1, VOCAB], F32)
        nc.vector.memset(ones1_sb[:], 1.0)
        p645_sb = consts.tile([128, 1], F32)
        nc.vector.memset(p645_sb[:], 64.5)

        # ---------------- MLP table: tbl[v] = silu(emb @ w1 + b1) @ w2 + b2
        embT_sb = consts.tile([L0, VOCAB], F32)
        nc.sync.dma_start(embT_sb[:], embT_d[:])
        w1_sb = consts.tile([L0, H], F32)
        nc.sync.dma_start(w1_sb[:], w1_d[:])
        b1_sb = consts.tile([H, 1], F32)
        nc.sync.dma_start(b1_sb[:], b1_d[:])
        w2_sb = consts.tile([H, 1], F32)
        nc.sync.dma_start(w2_sb[:], w2_d[:])
        b2_sb = consts.tile([1, 1], F32)
        nc.sync.dma_start(b2_sb[:], b2_d[:])

        # bounce matmul inputs through DVE so each matmul waits on one sem
        embT_c = consts.tile([L0, VOCAB], F32)
        nc.vector.tensor_copy(embT_c[:], embT_sb[:])
        w1_c = consts.tile([L0, H], F32)
        nc.vector.tensor_copy(w1_c[:], w1_sb[:])
        w2_c = consts.tile([H, 1], F32)
        nc.vector.tensor_copy(w2_c[:], w2_sb[:])
        b2_c = consts.tile([1, 1], F32)
        nc.vector.tensor_copy(b2_c[:], b2_sb[:])

        h1p = psum_s.tile([H, VOCAB], F32, tag="h1p")
        nc.tensor.matmul(h1p[:], w1_c[:], embT_c[:], start=True, stop=True)
        hts = consts.tile([H, VOCAB], F32)
        nc.scalar.activation(
            hts[:], h1p[:], mybir.ActivationFunctionType.Silu, bias=b1_sb[:, 0:1]
        )
        hts_c = consts.tile([H, VOCAB], F32)
        nc.vector.tensor_copy(hts_c[:], hts[:])
        t1p = psum_s.tile([VOCAB, 1], F32, tag="t1p")
        nc.tensor.matmul(t1p[:], hts_c[:], w2_c[:], start=True, stop=False)
        nc.tensor.matmul(t1p[:], ones1_sb[:], b2_c[:], start=False, stop=True)
        tbl_sb = consts.tile([VOCAB, 1], F32)
        nc.vector.tensor_copy(tbl_sb[:], t1p[:])

        # ---------------- out gather: outv[p, t] = tbl[z[t*128+p]]
        zf_sb = consts.tile([1, NT * 128], F32)
        nc.sync.dma_start(zf_sb[:], zf_d[:])
        zb = consts.tile([VOCAB, NT * 128], F32)
        nc.gpsimd.partition_broadcast(zb[:], zf_sb[0:1, :])
        oh = consts.tile([VOCAB, NT * 128], F32)
        nc.vector.tensor_scalar(
            oh[:], zb[:], iota_sb[:, 0:1], None, mybir.AluOpType.is_equal
        )
        outs_sb = consts.tile([128, NT], F32)
        ovp = psum_o.tile([128, NT], F32, tag="ovp")
        for t in range(NT):
            nc.tensor.matmul(
                ovp[:, t : t + 1],
                oh[:, t * 128 : (t + 1) * 128],
                tbl_sb[:],
                start=True,
                stop=True,
            )
        nc.scalar.copy(outs_sb[:], ovp[:])
        nc.sync.dma_start(outv_d[:], outs_sb[:])

        # ---------------- adjacency blocks (per slot, mixed sizes)
        GPB = max(profile)
        coff = 0
        noff = 0
        boff = 0
        for g, GP in enumerate(profile):
            RT = -(-GP // 128)
            posT_sb = gwork.tile([3, GPB], F32, tag="posT")
            nc.sync.dma_start(posT_sb[:, :GP], posT_d[:, coff : coff + GP])
            posT2_sb = gwork.tile([3, GPB], F32, tag="posT2")
            nc.vector.tensor_mul(posT2_sb[:, :GP], posT_sb[:, :GP], posT_sb[:, :GP])
            # 2*posT so the G matmul yields 2G directly (band absorbs rounding)
            posTx2_sb = gwork.tile([3, GPB], F32, tag="posTx2")
            nc.vector.tensor_scalar(
                posTx2_sb[:, :GP], posT_sb[:, :GP], 2.0, None, mybir.AluOpType.mult
            )
            # sqj[p, j] = |pos_j|^2 for every partition p (ones-matmul bcast)
            sqj_ps = psum_g.tile([128, GPB], F32, tag="sqj")
            nc.tensor.matmul(
                sqj_ps[:, :GP], ones3_sb[:], posT2_sb[:, :GP], start=True, stop=True
            )
            sqj_sb = gwork.tile([128, GPB], F32, tag="sqj_sb")
            nc.scalar.copy(sqj_sb[:, :GP], sqj_ps[:, :GP])
            # per-row norms |pos_i|^2 as per-partition scalars
            pn = gwork.tile([128, MAXRT, 3], F32, tag="pn")
            nc.sync.dma_start(
                pn[:, :RT, :],
                posn_d[noff : noff + RT * 128].rearrange("(r p) c -> p r c", p=128),
            )
            p2 = gwork.tile([128, MAXRT, 3], F32, tag="p2")
            nc.vector.tensor_mul(p2[:, :RT, :], pn[:, :RT, :], pn[:, :RT, :])
            sq_sb = gwork.tile([128, MAXRT], F32, tag="sq")
            nc.vector.reduce_sum(
                sq_sb[:, :RT], p2[:, :RT, :], axis=mybir.AxisListType.X
            )

            stage = gwork.tile([128, MAXRT, GPB], U8, tag="stage")
            stage2 = gwork.tile([128, MAXRT, GPB], U8, tag="stage2")
            for r in range(RT):
                M = min(128, GP - r * 128)
                rows = slice(r * 128, r * 128 + M)
                gm = psum.tile([128, GPB], F32, tag="gm")
                nc.tensor.matmul(
                    gm[:M, :GP],
                    posTx2_sb[:, rows],
                    posT_sb[:, :GP],
                    start=True,
                    stop=True,
                )
                d2_sb = work.tile([128, GPB], F32, tag="d2")
                nc.vector.scalar_tensor_tensor(
                    d2_sb[:M, :GP],
                    sqj_sb[:M, :GP],
                    sq_sb[:M, r : r + 1],
                    gm[:M, :GP],
                    op0=mybir.AluOpType.add,
                    op1=mybir.AluOpType.subtract,
                )
                nc.gpsimd.tensor_scalar(
                    stage[:M, r, :GP], d2_sb[:M, :GP], 63.5, None,
                    mybir.AluOpType.is_lt,
                )
                # hi plane on ACT: sign(64.5 - d2) -> u8 {1: lt, 0: eq, 255: gt}
                nc.scalar.activation(
                    stage2[:M, r, :GP],
                    d2_sb[:M, :GP],
                    mybir.ActivationFunctionType.Sign,
                    bias=p645_sb[:M, 0:1],
                    scale=-1.0,
                )
            nc.sync.dma_start(
                adj_d[boff : boff + RT * 128 * GP].rearrange(
                    "(r p c) -> p r c", p=128, c=GP
                ),
                stage[:, :RT, :GP],
            )
            nc.sync.dma_start(
                adj2_d[boff : boff + RT * 128 * GP].rearrange(
                    "(r p c) -> p r c", p=128, c=GP
                ),
                stage2[:, :RT, :GP],
            )
            coff += GP
            noff += RT * 128
            boff += RT * 128 * GP

    nc.finalize()
    return nc


def _cpu_xla_d2(a, b):
    """Bit-exact emulation of the reference's fp32 d2 on CPU XLA for pair
    arrays a, b of shape [k, 3] (float32)."""
    a = a.astype(np.float32)
    b = b.astype(np.float32)
    a64 = a.astype(np.float64)
    b64 = b.astype(np.float64)
    g = (a[:, 0] * b[:, 0]).astype(np.float32)
    g = (a64[:, 1] * b64[:, 1] + g.astype(np.float64)).astype(np.float32)
    g = (a64[:, 2] * b64[:, 2] + g.astype(np.float64)).astype(np.float32)

    def sq(v):
        return ((v[:, 0] * v[:, 0] + v[:, 1] * v[:, 1]).astype(np.float32)
                + v[:, 2] * v[:, 2]).astype(np.float32)

    t1 = (sq(a) + sq(b)).astype(np.float32)
    return (t1 - (np.float32(2.0) * g).astype(np.float32)).astype(np.float32)


def _prepare(z, batch, pos, emb, w1, b1, w2, b2):
    z = np.asarray(z)
    batch = np.asarray(batch)
    pos = np.ascontiguousarray(np.asarray(pos), dtype=np.float32)
    emb = np.asarray(emb, dtype=np.float32)
    w1 = np.asarray(w1, dtype=np.float32)
    b1 = np.asarray(b1, dtype=np.float32)
    w2 = np.asarray(w2, dtype=np.float32)
    b2 = np.asarray(b2, dtype=np.float32)

    N = z.shape[0]
    VOCAB, L0 = emb.shape
    H = w1.shape[1]

    ids = np.arange(B_GRAPHS)
    starts = np.searchsorted(batch, ids, "left").astype(np.int64)
    ends = np.searchsorted(batch, ids, "right").astype(np.int64)
    sizes = (ends - starts).astype(np.int64)

    # slot profile: slot s (same on every core) holds graphs ranked
    # [s*8, s*8+8) by size, so its width is the max size in that rank band
    order = np.argsort(-sizes, kind="stable")  # graph ids, biggest first
    profile = tuple(
        int(max(1, sizes[order[s * N_CORES]])) for s in range(GPC)
    )

    # assignment: slot (core m, slot s) <- order[s*N_CORES + m]
    slot_graph = np.full((N_CORES, GPC), -1, dtype=np.int64)
    for s in range(GPC):
        for m in range(N_CORES):
            slot_graph[m, s] = order[s * N_CORES + m]

    core_cnt = np.array(
        [int(sizes[slot_graph[m]].sum()) for m in range(N_CORES)], dtype=np.int64
    )
    NP = int(max(128, -(-int(core_cnt.max()) // 128) * 128))
    NT = NP // 128
    TOTC = sum(profile)

    rts = [-(-gp // 128) for gp in profile]
    TOTN = sum(rt * 128 for rt in rts)
    in_maps = []
    for m in range(N_CORES):
        posT = np.full((3, TOTC), PAD_POS, dtype=np.float32)
        posn = np.full((TOTN, 3), PAD_POS, dtype=np.float32)
        zc = np.zeros(NP, dtype=np.float32)
        coff = 0
        noff = 0
        zoff = 0
        for s, GP in enumerate(profile):
            g = int(slot_graph[m, s])
            sg, eg = int(starts[g]), int(ends[g])
            ng = eg - sg
            posn[noff : noff + ng] = pos[sg:eg]
            posT[:, coff : coff + ng] = pos[sg:eg].T
            zc[zoff : zoff + ng] = z[sg:eg]
            coff += GP
            noff += rts[s] * 128
            zoff += ng
        in_maps.append(
            {
                "posT": posT,
                "posn": posn,
                "zf": zc.reshape(1, NP),
                "embT": np.ascontiguousarray(emb.T),
                "w1": w1,
                "b1": b1.reshape(H, 1),
                "w2": w2.reshape(H, 1),
                "b2": b2.reshape(1, 1),
            }
        )

    meta = {
        "N": N,
        "profile": profile,
        "NT": NT,
        "VOCAB": VOCAB,
        "L0": L0,
        "H": H,
        "starts": starts,
        "ends": ends,
        "slot_graph": slot_graph,
        "pos": pos,
    }
    return in_maps, meta


def _get_nc(meta):
    key = (meta["profile"], meta["NT"], meta["VOCAB"], meta["L0"], meta["H"])
    nc = _NC_CACHE.get(key)
    if nc is None:
        nc = _build_nc(*key)
        _NC_CACHE[key] = nc
    return nc


def _assemble(results, meta):
    N = meta["N"]
    starts, ends = meta["starts"], meta["ends"]
    slot_graph = meta["slot_graph"]
    profile = meta["profile"]
    pos = meta["pos"]

    out = np.empty((N, 1), dtype=np.float32)
    adj = np.zeros((N, N), dtype=bool)
    for m in range(N_CORES):
        r = results[m]
        outv = r["outv"].T.ravel()
        boff = 0
        zoff = 0
        for s, GP in enumerate(profile):
            RTs = -(-GP // 128)
            g = int(slot_graph[m, s])
            sg, eg = int(starts[g]), int(ends[g])
            n = eg - sg
            lo_full = r["adj_lo"][boff : boff + RTs * 128 * GP].reshape(
                RTs * 128, GP
            )
            hi_full = r["adj_hi"][boff : boff + RTs * 128 * GP].reshape(
                RTs * 128, GP
            )
            boff += RTs * 128 * GP
            out[sg:eg, 0] = outv[zoff : zoff + n]
            zoff += n
            if n == 0:
                continue
            blk = lo_full[:n, :n].astype(bool)      # d2 < 63.5
            hi = hi_full[:n, :n] == 1               # d2 < 64.5
            bi, bj = np.nonzero(blk != hi)          # near-threshold pairs
            if bi.size:
                P = pos[sg:eg]
                d2 = _cpu_xla_d2(P[bi], P[bj])
                blk[bi, bj] = d2 < np.float32(RADIUS2)
            np.fill_diagonal(blk, False)
            adj[sg:eg, sg:eg] = blk
    return out, adj


def kernel(**inputs):
    in_maps, meta = _prepare(**inputs)
    nc = _get_nc(meta)
    res = run_bass_kernel_spmd(nc, in_maps, list(range(N_CORES)))
    return _assemble(res.results, meta)
